# revision 3
# baseline (speedup 1.0000x reference)
"""Trainium2 Bass kernel for NonLocalAttention (fused 1x1 convs + spatial softmax attention).

Reference computation (N=2, C=64, FC=64, CR=32, H=W=96, HW=9216):
    q = relu(wq @ x + bq)          [N, 32, HW]
    k = relu(wk @ fm + bk)         [N, 32, HW]
    v = relu(wa @ fm + ba)         [N, 64, HW]
    s = softmax(q^T k, axis=keys)  [N, HW, HW]
    o = s @ v^T                    [N, HW, 64]
    out = relu(wo @ [x; o^T] + bo) [N, 64, HW]

Sharding: 8 cores = batch(2) x query-rows(4).  Each core handles 2304 query
pixels of one batch element and needs the full fusionmap of that batch.

Per-core kernel (flash-style, score never goes to HBM):
  - score is computed TRANSPOSED: st[key, q] = k^T q via row-packed (K=32)
    bf16 matmuls, 3 key-tiles of 128 at a time into 3 PSUM banks (the three
    row-group matmuls run concurrently in the PE array).
  - scores are >= 0 (q and k are relu'd), bounded ~6.5, so softmax needs no
    max subtraction.  exp uses a fixed bias of -2 so exp(s-2) in [0.14, ~90]
    fits fp8e4m3 comfortably (max finite 240); the constant factor cancels in
    the softmax ratio.
  - exp is split between BOTH elementwise engines to halve the exp wall time
    (it is the kernel bottleneck at ~170us if done on ScalarE alone):
      * ScalarE: true exp activation, PSUM -> fp8e4 SBUF, query cols [0, R)
      * VectorE: Schraudolph fast exp - one tensor_scalar (x*A + B) with
        round-to-nearest conversion to uint8; the byte IS the fp8e4m3 bit
        pattern of 2^((i-56)/8) ~= exp(x-2).  Query cols [R, qn).
    The split is by query column, so each query's full softmax row (numerator
    and denominator) uses one engine consistently and the per-engine
    multiplicative error largely cancels in the ratio.
  - mm2 contracts keys with fp8 DoubleRow matmuls: each matmul processes TWO
    key tiles (virtual 128x256 array) with lhsT = [vT | 1] pairs, so PSUM row
    64 accumulates the softmax denominator for free and mm2 PE time halves.
  - normalize with reciprocal + DMA partition-broadcast, then the output
    1x1 conv (wo) is fused in, relu, DMA out.
"""

import sys

sys.path.insert(0, "/opt/trn_rl_repo")

from contextlib import ExitStack

import ml_dtypes
import numpy as np

import concourse.bacc as bacc
import concourse.bass as bass
import concourse.tile as tile
from concourse import mybir
from concourse import bass_utils

C = 64
FC = 64
CR = 32
N = 2
H = W = 96
HW = H * W            # 9216
NCORES = 8
QPC = HW // 4         # queries per core = 2304
NKT = HW // 128       # 72 key tiles
G = 3                 # row-packing group (3 key tiles concurrently)
NJ = NKT // G         # 24 key-tile groups
NJJ = NJ // 2         # 12 super-steps (2 j's = 6 key tiles = 3 DoubleRow pairs)
QCHUNKS = [(0, 512), (512, 512), (1024, 512), (1536, 512), (2048, 256)]

F32 = mybir.dt.float32
F32R = mybir.dt.float32r
BF16 = mybir.dt.bfloat16
F8 = mybir.dt.float8e4
U8 = mybir.dt.uint8
ATT = BF16            # q/k dtype for mm1

# exp split point: ScalarE does query cols [0, R) of each 512-wide chunk,
# VectorE (Schraudolph) does [R, 512).  Scaled proportionally for the 256
# chunk.  Tuned so both engines' per-step time balances.
R_SPLIT = 320

# Schraudolph constants: byte = round(A*s + B) is the fp8e4m3 bit pattern of
# ~exp(s-2).  c = -0.42 centers the log-ratio error (spread ~0.146, on par
# with plain e4m3 quantization of exact exp at 0.118).
SCHR_A = 8.0 / np.log(2.0)
SCHR_B = 56.0 - 2.0 * SCHR_A - 0.42


def build_bass():
    nc = bacc.Bacc(
        "TRN2", target_bir_lowering=False, debug=False, num_devices=NCORES
    )

    x_aug = nc.dram_tensor("x_aug", [C + 1, QPC], F32R, kind="ExternalInput")
    fm_aug = nc.dram_tensor("fm_aug", [FC + 1, HW], BF16, kind="ExternalInput")
    wq_aug = nc.dram_tensor("wq_aug", [C + 1, CR], F32R, kind="ExternalInput")
    wk_aug = nc.dram_tensor("wk_aug", [FC + 1, CR], BF16, kind="ExternalInput")
    wa_aug = nc.dram_tensor("wa_aug", [FC + 1, C + 1], BF16, kind="ExternalInput")
    wox_aug = nc.dram_tensor("wox_aug", [C + 1, C], F32R, kind="ExternalInput")
    woa_t = nc.dram_tensor("woa_t", [C, C], F32R, kind="ExternalInput")
    out_d = nc.dram_tensor("out_c", [C, QPC], F32, kind="ExternalOutput")

    with tile.TileContext(nc) as tc, ExitStack() as ctx:
        consts = ctx.enter_context(tc.tile_pool(name="consts", bufs=1))
        stp = ctx.enter_context(tc.tile_pool(name="stp", bufs=2))
        wk_pool = ctx.enter_context(tc.tile_pool(name="work", bufs=3))
        # PSUM: 2x3 banks score tiles + 2x1 bank shared acc/fin/v-conv = 8
        psA = ctx.enter_context(tc.tile_pool(name="psA", bufs=2, space="PSUM"))
        psO = ctx.enter_context(tc.tile_pool(name="psO", bufs=2, space="PSUM"))

        # ---- constants / inputs in SBUF ----
        # fusionmap in 4 quarters so k-conv starts before the full DMA lands
        NQT = 4
        HWQ = HW // NQT  # 2304 = 18 key tiles per quarter
        FMq = [
            consts.tile([FC + 1, HWQ], BF16, tag=f"fm{p}", name=f"FM{p}")
            for p in range(NQT)
        ]
        XA = consts.tile([C + 1, QPC], F32R)         # x chunk + ones row
        WQ = consts.tile([C + 1, CR], F32R)
        WK = consts.tile([FC + 1, CR], BF16)
        WA = consts.tile([FC + 1, C + 1], BF16)
        WOX = consts.tile([C + 1, C], F32R)
        WOA = consts.tile([C, C], F32R)
        nc.sync.dma_start(WK[:], wk_aug.ap())
        for p in range(NQT):
            nc.sync.dma_start(FMq[p][:], fm_aug.ap()[:, p * HWQ : (p + 1) * HWQ])
        nc.sync.dma_start(XA[:], x_aug.ap())
        nc.sync.dma_start(WQ[:], wq_aug.ap())
        nc.sync.dma_start(WA[:], wa_aug.ap())
        nc.sync.dma_start(WOX[:], wox_aug.ap())
        nc.sync.dma_start(WOA[:], woa_t.ap())

        def fm_kt(kt):  # [65, 128] slice of fusionmap for key tile kt
            p, i = divmod(kt, 18)
            return FMq[p][:, 128 * i : 128 * (i + 1)]

        # KR: k channels row-packed: partitions 32g..32g+31 hold key tile
        # kt=3j+g at free block j; split in j-quarters for earlier start.
        # QR: per-chunk tiles, q replicated on partition groups 0..2.
        KRq = [
            consts.tile([128, NJ // NQT, 128], ATT, tag=f"kr{p}", name=f"KR{p}")
            for p in range(NQT)
        ]
        QRc = [
            consts.tile([128, qn], ATT, tag=f"qr{ci}", name=f"QR{ci}")
            for ci, (q0, qn) in enumerate(QCHUNKS)
        ]
        # VT: [keys(128), kt, C+1] in fp8e4, padded to 80 so a DoubleRow
        # weight pair AP [128, 2, 65] has a 16-byte-multiple pair stride.
        # Column C is 1.0 straight out of the v-conv (wa is augmented with a
        # ones column), so mm2 accumulates the softmax denominator in PSUM
        # row C for free.
        NVR = NKT // 4
        VTr = [
            consts.tile([128, 4, 80], F8, tag=f"vt{r}", name=f"VT{r}")
            for r in range(NVR)
        ]
        ones1 = consts.tile([1, C], F32R)
        nc.vector.memset(ones1[:].bitcast(F32), 1.0)
        BIASM2 = consts.tile([128, 1], F32)
        nc.vector.memset(BIASM2[:], -2.0)

        # Preload the exp table set (~2.7us) off the critical path, before
        # the first real exp.
        dummy = wk_pool.tile([1, 1], F32, tag="dummy", name="dummy")
        nc.scalar.activation(
            dummy[:], ones1[0:1, 0:1].bitcast(F32),
            mybir.ActivationFunctionType.Exp,
        )

        # ---- phase 1: q / k convs (emitted interleaved with phase 2) ----
        # fp32r matmuls must write PSUM partition 0, so produce plain q/k at
        # partitions 0-31 and rearrange into the packed layouts via DMA.
        # k quarter p: plain [32, HW/4] as 6 x 384-wide chunks, then
        # interleave kt%3 -> partition group via DMA
        Ksq = [
            consts.tile([CR, HWQ], ATT, tag=f"ks{p}", name=f"Ks{p}")
            for p in range(NQT)
        ]

        def k_chunk(p, c):
            ps = psA.tile([128, G, 512], F32, tag="sc", name="kps")
            nc.tensor.matmul(
                ps[0:CR, 0, 0:384], WK[:], FMq[p][:, 384 * c : 384 * (c + 1)]
            )
            nc.vector.tensor_scalar_max(
                Ksq[p][:, 384 * c : 384 * (c + 1)], ps[0:CR, 0, 0:384], 0.0
            )
            if c == HWQ // 384 - 1:
                Ksv = Ksq[p].rearrange("p (j g c) -> p j g c", g=G, c=128)
                for g in range(G):
                    nc.sync.dma_start(
                        KRq[p][32 * g : 32 * g + 32, :, :], Ksv[:, :, g, :]
                    )

        # q chunk ci: relu into QRc[ci][0:32], then replicate to groups 1, 2
        def q_chunk(ci):
            q0, qn = QCHUNKS[ci]
            ps = psA.tile([128, G, 512], F32, tag="sc", name="qps")
            nc.tensor.matmul(
                ps[0:CR, 0, 0:qn], WQ[:], XA[:, q0 : q0 + qn]
            )
            nc.vector.tensor_scalar_max(
                QRc[ci][0:CR, 0:qn], ps[0:CR, 0, 0:qn], 0.0
            )
            nc.sync.dma_start(QRc[ci][32:64, 0:qn], QRc[ci][0:32, 0:qn])
            nc.sync.dma_start(QRc[ci][64:96, 0:qn], QRc[ci][0:32, 0:qn])

        # all k and q convs precede the main loop (the first ~60us run at
        # half PE clock regardless; interleaving convs into the attention
        # loop measured slower than running them up front)
        for p in range(NQT):
            for c in range(HWQ // 384):
                k_chunk(p, c)
        for ci in range(len(QCHUNKS)):
            q_chunk(ci)

        # v^T conv round r: out[key, 0:64] = relu'd v, out[key, 64] = 1.0
        # (from the wa ones block) for key tiles 4r..4r+3 into one PSUM bank,
        # relu'd + converted to fp8 in one DVE op into the padded VT layout.
        def v_round(r):
            ps = psO.tile([128, 512], F32, tag="acc")
            for i in range(4):
                kt = 4 * r + i
                nc.tensor.matmul(
                    ps[:, (C + 1) * i : (C + 1) * (i + 1)], fm_kt(kt), WA[:]
                )
            nc.vector.tensor_scalar_max(
                VTr[r][:, 0:4, 0 : C + 1],
                ps[:, 0 : 4 * (C + 1)].rearrange("p (a b) -> p a b", b=C + 1),
                0.0,
            )

        # ---- phase 2: attention + output conv, per query chunk ----
        # The normalize + output conv of chunk qc is emitted inside chunk
        # qc+1's loop so the PE FIFO is not blocked at chunk boundaries.
        def finalize(acc, q0, qn):
            # row C of acc = sum_k exp(score); reciprocal, then broadcast to
            # 64 partitions with a K=1 matmul against a ones column.
            recip1 = wk_pool.tile([1, 512], F32R, tag="recip1", name="recip1")
            with nc.allow_low_precision(reason="fp32r reciprocal, ~tf32 ok"):
                nc.vector.reciprocal(recip1[:, 0:qn], acc[C : C + 1, 0:qn])
            rb_ps = psA.tile([128, G, 512], F32, tag="sc", name="rb_ps")
            nc.tensor.matmul(rb_ps[0:C, 0, 0:qn], ones1[:], recip1[:, 0:qn])
            rbS = wk_pool.tile([C, 512], F32, tag="rbS", name="rbS")
            nc.vector.tensor_copy(rbS[:, 0:qn], rb_ps[0:C, 0, 0:qn])
            attn = wk_pool.tile([C, 512], F32R, tag="attn", name="attn")
            nc.vector.tensor_mul(attn[:, 0:qn], acc[0:C, 0:qn], rbS[:, 0:qn])
            # out = relu(wo_x @ x + wo_a @ attn + bo)
            fin = psO.tile([128, 512], F32, tag="acc", name="fin")
            nc.tensor.matmul(
                fin[0:C, 0:qn], WOX[:], XA[:, q0 : q0 + qn],
                start=True, stop=False,
            )
            nc.tensor.matmul(
                fin[0:C, 0:qn], WOA[:], attn[:, 0:qn],
                start=False, stop=True,
            )
            outs = wk_pool.tile([C, 512], F32, tag="outs", name="outs")
            nc.vector.tensor_scalar_max(outs[:, 0:qn], fin[0:C, 0:qn], 0.0)
            nc.sync.dma_start(out_d.ap()[:, q0 : q0 + qn], outs[:, 0:qn])

        # Flat software-pipelined emission over t = (chunk, j).  Each j emits
        # the mm1 trio and the two exp halves; DoubleRow mm2 pairs for a
        # completed super-step (2 j's) are drained with ~1-step lag so the PE
        # FIFO never waits on a just-issued exp.
        NT = len(QCHUNKS) * NJ
        accs = [None] * len(QCHUNKS)
        pend_pairs = []   # (qi, jj, p, st2_tile, qn)
        pend_fin = None
        vr_next = 0

        def emit_pair(qi, jj, p, st2, qn):
            kt = 6 * jj + 2 * p
            r, i = divmod(kt, 4)
            nc.tensor.matmul(
                accs[qi][0 : C + 1, 0:qn],
                VTr[r][:, i : i + 2, 0 : C + 1],
                st2[:, 2 * p : 2 * p + 2, 0:qn],
                start=(jj == 0 and p == 0),
                stop=(jj == NJJ - 1 and p == 2),
                perf_mode=mybir.MatmulPerfMode.DoubleRow,
            )

        st2_cur = None
        for t in range(NT):
            qi, j = divmod(t, NJ)
            jj = j // 2
            q0, qn = QCHUNKS[qi]
            if j == 0:
                accs[qi] = psO.tile([128, 512], F32, tag="acc", name="acc")
            if qi == 0:
                # remaining k-conv quarters, front-loaded so each quarter's
                # interleave DMA lands well before its first mm1 consumer
                # (quarter p needed from j = 6p)
                KSCHED = [3, 3, 2, 2, 2, 2, 2, 2]
                if j < len(KSCHED):
                    base = 6 + sum(KSCHED[:j])
                    for kc in range(base, base + KSCHED[j]):
                        k_chunk(kc // 6, kc % 6)
                # remaining q-conv chunks, well before their chunk starts
                if j in (2, 4, 6, 8):
                    q_chunk(j // 2)
            # keep v-conv a little ahead of the mm2 consumer on chunk 0
            while vr_next < NVR and (qi > 0 or 4 * vr_next <= 3 * j + 6):
                v_round(vr_next)
                vr_next += 1
            sc = psA.tile([128, G, 512], F32, tag="sc")
            jq, jjq = divmod(j, NJ // NQT)
            for g in range(G):
                nc.tensor.matmul(
                    sc[:, g, 0:qn],
                    KRq[jq][32 * g : 32 * g + 32, jjq, :],
                    QRc[qi][32 * g : 32 * g + 32, 0:qn],
                )
            if j % 2 == 0:
                st2_cur = stp.tile([128, 6, 512], F8, tag="st")
            g0 = 3 * (j % 2)
            rc = (R_SPLIT * qn) // 512
            nc.scalar.activation(
                st2_cur[:, g0 : g0 + 3, 0:rc],
                sc[:, :, 0:rc],
                mybir.ActivationFunctionType.Exp,
                bias=BIASM2[:],
            )
            nc.vector.tensor_scalar(
                st2_cur[:, g0 : g0 + 3, rc:qn].bitcast(U8),
                sc[:, :, rc:qn],
                float(SCHR_A), float(SCHR_B),
                mybir.AluOpType.mult, mybir.AluOpType.add,
            )
            if j % 2 == 1:
                for p in range(3):
                    pend_pairs.append((qi, jj, p, st2_cur, qn))
            # drain mm2 pairs with a ~1-step lag (do not emit pairs of the
            # super-step whose exp was just issued this t)
            budget = 2 if j % 2 == 0 else 1
            while pend_pairs and budget > 0:
                pqi, pjj, pp, pst2, pqn = pend_pairs[0]
                if pqi == qi and pjj == jj:
                    break
                emit_pair(*pend_pairs.pop(0))
                budget -= 1
            if j == 4 and pend_fin is not None:
                finalize(*pend_fin)
                pend_fin = None
            if j == NJ - 1:
                pend_fin = (accs[qi], q0, qn)
        for pair in pend_pairs:
            emit_pair(*pair)
        finalize(*pend_fin)

    nc.compile()
    return nc


_NC_CACHE = None


def _get_nc():
    global _NC_CACHE
    if _NC_CACHE is None:
        _NC_CACHE = build_bass()
    return _NC_CACHE


def make_in_maps(x, fusionmap, wq, bq, wk, bk, wa, ba, wo, bo):
    x = np.asarray(x, np.float32)
    fm = np.asarray(fusionmap, np.float32)
    xf = x.reshape(N, C, HW)
    fmf = fm.reshape(N, FC, HW)
    ones_hw = np.ones((1, HW), np.float32)
    wq_aug = np.concatenate(
        [np.asarray(wq).T, np.asarray(bq)[None, :]], 0
    ).astype(np.float32)
    wk_aug = np.concatenate(
        [np.asarray(wk).T, np.asarray(bk)[None, :]], 0
    ).astype(ml_dtypes.bfloat16)
    # [wa^T | 0; ba | 1]: columns C..2C-1 evaluate to exactly 1.0 after the
    # conv (ones row of fm_aug x ones), giving mm2 its denominator columns.
    wa_blk = np.concatenate([np.asarray(wa).T, np.asarray(ba)[None, :]], 0)
    ones_blk = np.concatenate(
        [np.zeros((FC, 1), np.float32), np.ones((1, 1), np.float32)], 0
    )
    wa_aug = np.concatenate([wa_blk, ones_blk], 1).astype(ml_dtypes.bfloat16)
    wo = np.asarray(wo, np.float32)
    wox_aug = np.concatenate(
        [wo[:, :C].T, np.asarray(bo)[None, :]], 0
    ).astype(np.float32)
    woa_t = np.ascontiguousarray(wo[:, C:].T).astype(np.float32)

    in_maps = []
    for core in range(NCORES):
        n, c = divmod(core, 4)
        x_chunk = xf[n][:, c * QPC : (c + 1) * QPC]
        x_aug = np.concatenate([x_chunk, ones_hw[:, :QPC]], 0)
        fm_aug = np.concatenate([fmf[n], ones_hw], 0).astype(ml_dtypes.bfloat16)
        in_maps.append(
            {
                "x_aug": np.ascontiguousarray(x_aug),
                "fm_aug": np.ascontiguousarray(fm_aug),
                "wq_aug": wq_aug,
                "wk_aug": wk_aug,
                "wa_aug": wa_aug,
                "wox_aug": wox_aug,
                "woa_t": woa_t,
            }
        )
    return in_maps


def run(in_maps, trace=False, tmpdir=None):
    nc = _get_nc()
    return bass_utils.run_bass_kernel_spmd(
        nc,
        in_maps,
        core_ids=list(range(NCORES)),
        trace=trace,
        tmpdir=tmpdir,
    )


def kernel(**inputs):
    in_maps = make_in_maps(**inputs)
    res = run(in_maps)
    out = np.empty((N, C, HW), np.float32)
    for core in range(NCORES):
        n, c = divmod(core, 4)
        out[n][:, c * QPC : (c + 1) * QPC] = res.results[core]["out_c"]
    return out.reshape(N, C, H, W)


if __name__ == "__main__":
    import reference

    inputs = {k: np.asarray(v) for k, v in reference.setup_inputs().items()}
    got = kernel(**inputs)
    print("kernel output", got.shape, got.dtype)


# revision 5
# speedup vs baseline: 1.0053x; 1.0053x over previous
"""Trainium2 Bass kernel for NonLocalAttention (fused 1x1 convs + spatial softmax attention).

Reference computation (N=2, C=64, FC=64, CR=32, H=W=96, HW=9216):
    q = relu(wq @ x + bq)          [N, 32, HW]
    k = relu(wk @ fm + bk)         [N, 32, HW]
    v = relu(wa @ fm + ba)         [N, 64, HW]
    s = softmax(q^T k, axis=keys)  [N, HW, HW]
    o = s @ v^T                    [N, HW, 64]
    out = relu(wo @ [x; o^T] + bo) [N, 64, HW]

Sharding: 8 cores = batch(2) x query-rows(4).  Each core handles 2304 query
pixels of one batch element and needs the full fusionmap of that batch.

Per-core kernel (flash-style, score never goes to HBM):
  - score is computed TRANSPOSED: st[key, q] = k^T q via row-packed (K=32)
    bf16 matmuls, 3 key-tiles of 128 at a time into 3 PSUM banks (the three
    row-group matmuls run concurrently in the PE array).
  - scores are >= 0 (q and k are relu'd), bounded ~6.5, so softmax needs no
    max subtraction.  exp uses a fixed bias of -2 so exp(s-2) in [0.14, ~90]
    fits fp8e4m3 comfortably (max finite 240); the constant factor cancels in
    the softmax ratio.
  - exp is split between BOTH elementwise engines to halve the exp wall time
    (it is the kernel bottleneck at ~170us if done on ScalarE alone):
      * ScalarE: true exp activation, PSUM -> fp8e4 SBUF, query cols [0, R)
      * VectorE: Schraudolph fast exp - one tensor_scalar (x*A + B) with
        round-to-nearest conversion to uint8; the byte IS the fp8e4m3 bit
        pattern of 2^((i-56)/8) ~= exp(x-2).  Query cols [R, qn).
    The split is by query column, so each query's full softmax row (numerator
    and denominator) uses one engine consistently and the per-engine
    multiplicative error largely cancels in the ratio.
  - mm2 contracts keys with fp8 DoubleRow matmuls: each matmul processes TWO
    key tiles (virtual 128x256 array) with lhsT = [vT | 1] pairs, so PSUM row
    64 accumulates the softmax denominator for free and mm2 PE time halves.
  - normalize with reciprocal + DMA partition-broadcast, then the output
    1x1 conv (wo) is fused in, relu, DMA out.
"""

import sys

sys.path.insert(0, "/opt/trn_rl_repo")

from contextlib import ExitStack

import ml_dtypes
import numpy as np

import concourse.bacc as bacc
import concourse.bass as bass
import concourse.tile as tile
from concourse import mybir
from concourse import bass_utils

C = 64
FC = 64
CR = 32
N = 2
H = W = 96
HW = H * W            # 9216
NCORES = 8
QPC = HW // 4         # queries per core = 2304
NKT = HW // 128       # 72 key tiles
G = 3                 # row-packing group (3 key tiles concurrently)
NJ = NKT // G         # 24 key-tile groups
NJJ = NJ // 2         # 12 super-steps (2 j's = 6 key tiles = 3 DoubleRow pairs)
QCHUNKS = [(0, 512), (512, 512), (1024, 512), (1536, 512), (2048, 256)]

F32 = mybir.dt.float32
F32R = mybir.dt.float32r
BF16 = mybir.dt.bfloat16
F8 = mybir.dt.float8e4
U8 = mybir.dt.uint8
ATT = BF16            # q/k dtype for mm1

# exp split point: ScalarE does query cols [0, R) of each 512-wide chunk,
# VectorE (Schraudolph) does [R, 512).  Scaled proportionally for the 256
# chunk.  Tuned so both engines' per-step time balances.
R_SPLIT = 320

# Schraudolph constants: byte = round(A*s + B) is the fp8e4m3 bit pattern of
# ~exp(s-2).  c = -0.42 centers the log-ratio error (spread ~0.146, on par
# with plain e4m3 quantization of exact exp at 0.118).
SCHR_A = 8.0 / np.log(2.0)
SCHR_B = 56.0 - 2.0 * SCHR_A - 0.42


def build_bass():
    nc = bacc.Bacc(
        "TRN2", target_bir_lowering=False, debug=False, num_devices=NCORES
    )

    x_aug = nc.dram_tensor("x_aug", [C + 1, QPC], F32R, kind="ExternalInput")
    fm_aug = nc.dram_tensor("fm_aug", [FC + 1, HW], BF16, kind="ExternalInput")
    wq_aug = nc.dram_tensor("wq_aug", [C + 1, CR], F32R, kind="ExternalInput")
    wk_aug = nc.dram_tensor("wk_aug", [FC + 1, CR], BF16, kind="ExternalInput")
    wa_aug = nc.dram_tensor("wa_aug", [FC + 1, C + 1], BF16, kind="ExternalInput")
    wox_aug = nc.dram_tensor("wox_aug", [C + 1, C], F32R, kind="ExternalInput")
    woa_t = nc.dram_tensor("woa_t", [C, C], F32R, kind="ExternalInput")
    out_d = nc.dram_tensor("out_c", [C, QPC], F32, kind="ExternalOutput")

    with tile.TileContext(nc) as tc, ExitStack() as ctx:
        consts = ctx.enter_context(tc.tile_pool(name="consts", bufs=1))
        stp = ctx.enter_context(tc.tile_pool(name="stp", bufs=2))
        wk_pool = ctx.enter_context(tc.tile_pool(name="work", bufs=3))
        # PSUM: 2x3 banks score tiles + 2x1 bank shared acc/fin/v-conv = 8
        psA = ctx.enter_context(tc.tile_pool(name="psA", bufs=2, space="PSUM"))
        psO = ctx.enter_context(tc.tile_pool(name="psO", bufs=2, space="PSUM"))

        # ---- constants / inputs in SBUF ----
        # fusionmap in 4 quarters so k-conv starts before the full DMA lands
        NQT = 4
        HWQ = HW // NQT  # 2304 = 18 key tiles per quarter
        FMq = [
            consts.tile([FC + 1, HWQ], BF16, tag=f"fm{p}", name=f"FM{p}")
            for p in range(NQT)
        ]
        XA = consts.tile([C + 1, QPC], F32R)         # x chunk + ones row
        WQ = consts.tile([C + 1, CR], F32R)
        WK = consts.tile([FC + 1, CR], BF16)
        WA = consts.tile([FC + 1, C + 1], BF16)
        WOX = consts.tile([C + 1, C], F32R)
        WOA = consts.tile([C, C], F32R)
        nc.sync.dma_start(WK[:], wk_aug.ap())
        for p in range(NQT):
            nc.sync.dma_start(FMq[p][:], fm_aug.ap()[:, p * HWQ : (p + 1) * HWQ])
        nc.sync.dma_start(XA[:], x_aug.ap())
        nc.sync.dma_start(WQ[:], wq_aug.ap())
        nc.sync.dma_start(WA[:], wa_aug.ap())
        nc.sync.dma_start(WOX[:], wox_aug.ap())
        nc.sync.dma_start(WOA[:], woa_t.ap())

        def fm_kt(kt):  # [65, 128] slice of fusionmap for key tile kt
            p, i = divmod(kt, 18)
            return FMq[p][:, 128 * i : 128 * (i + 1)]

        # KR: k channels row-packed: partitions 32g..32g+31 hold key tile
        # kt=3j+g at free block j; split in j-quarters for earlier start.
        # QR: per-chunk tiles, q replicated on partition groups 0..2.
        KRq = [
            consts.tile([128, NJ // NQT, 128], ATT, tag=f"kr{p}", name=f"KR{p}")
            for p in range(NQT)
        ]
        QRc = [
            consts.tile([128, qn], ATT, tag=f"qr{ci}", name=f"QR{ci}")
            for ci, (q0, qn) in enumerate(QCHUNKS)
        ]
        # VT: [keys(128), kt, C+1] in fp8e4, padded to 80 so a DoubleRow
        # weight pair AP [128, 2, 65] has a 16-byte-multiple pair stride.
        # Column C is 1.0 straight out of the v-conv (wa is augmented with a
        # ones column), so mm2 accumulates the softmax denominator in PSUM
        # row C for free.
        NVR = NKT // 4
        VTr = [
            consts.tile([128, 4, 80], F8, tag=f"vt{r}", name=f"VT{r}")
            for r in range(NVR)
        ]
        ones1 = consts.tile([1, C], F32R)
        nc.vector.memset(ones1[:].bitcast(F32), 1.0)
        BIASM2 = consts.tile([128, 1], F32)
        nc.vector.memset(BIASM2[:], -2.0)

        # Preload the exp table set (~2.7us) off the critical path, before
        # the first real exp.
        dummy = wk_pool.tile([1, 1], F32, tag="dummy", name="dummy")
        nc.scalar.activation(
            dummy[:], ones1[0:1, 0:1].bitcast(F32),
            mybir.ActivationFunctionType.Exp,
        )

        # PE warm-up: ~20 back-to-back dummy matmuls (~6us) while the input
        # DMAs land.  The PE HAM clock gate defaults to 1.2 GHz and only
        # un-throttles after a ~3.4us continuously-busy window; without this
        # the whole kernel can run at half PE clock.  Operands are an
        # uninitialized tile (never read downstream).
        warm_src = consts.tile([128, 512], BF16, name="warm_src")
        nc.gpsimd.memset(warm_src[:], 0.0)
        warm_ps = psO.tile([128, 512], F32, tag="acc", name="warm_ps")
        for _ in range(20):
            nc.tensor.matmul(warm_ps[:, 0:512], warm_src[:, 0:128], warm_src[:])

        # ---- phase 1: q / k convs (emitted interleaved with phase 2) ----
        # fp32r matmuls must write PSUM partition 0, so produce plain q/k at
        # partitions 0-31 and rearrange into the packed layouts via DMA.
        # k quarter p: plain [32, HW/4] as 6 x 384-wide chunks, then
        # interleave kt%3 -> partition group via DMA
        Ksq = [
            consts.tile([CR, HWQ], ATT, tag=f"ks{p}", name=f"Ks{p}")
            for p in range(NQT)
        ]

        def k_chunk(p, c):
            ps = psA.tile([128, G, 512], F32, tag="sc", name="kps")
            nc.tensor.matmul(
                ps[0:CR, 0, 0:384], WK[:], FMq[p][:, 384 * c : 384 * (c + 1)]
            )
            nc.vector.tensor_scalar_max(
                Ksq[p][:, 384 * c : 384 * (c + 1)], ps[0:CR, 0, 0:384], 0.0
            )
            if c == HWQ // 384 - 1:
                Ksv = Ksq[p].rearrange("p (j g c) -> p j g c", g=G, c=128)
                for g in range(G):
                    nc.sync.dma_start(
                        KRq[p][32 * g : 32 * g + 32, :, :], Ksv[:, :, g, :]
                    )

        # q chunk ci: relu into QRc[ci][0:32], then replicate to groups 1, 2
        def q_chunk(ci):
            q0, qn = QCHUNKS[ci]
            ps = psA.tile([128, G, 512], F32, tag="sc", name="qps")
            nc.tensor.matmul(
                ps[0:CR, 0, 0:qn], WQ[:], XA[:, q0 : q0 + qn]
            )
            nc.vector.tensor_scalar_max(
                QRc[ci][0:CR, 0:qn], ps[0:CR, 0, 0:qn], 0.0
            )
            nc.sync.dma_start(QRc[ci][32:64, 0:qn], QRc[ci][0:32, 0:qn])
            nc.sync.dma_start(QRc[ci][64:96, 0:qn], QRc[ci][0:32, 0:qn])

        # all k and q convs precede the main loop (the first ~60us run at
        # half PE clock regardless; interleaving convs into the attention
        # loop measured slower than running them up front)
        for p in range(NQT):
            for c in range(HWQ // 384):
                k_chunk(p, c)
        for ci in range(len(QCHUNKS)):
            q_chunk(ci)

        # v^T conv round r: out[key, 0:64] = relu'd v, out[key, 64] = 1.0
        # (from the wa ones block) for key tiles 4r..4r+3 into one PSUM bank,
        # relu'd + converted to fp8 in one DVE op into the padded VT layout.
        def v_round(r):
            ps = psO.tile([128, 512], F32, tag="acc")
            for i in range(4):
                kt = 4 * r + i
                nc.tensor.matmul(
                    ps[:, (C + 1) * i : (C + 1) * (i + 1)], fm_kt(kt), WA[:]
                )
            nc.vector.tensor_scalar_max(
                VTr[r][:, 0:4, 0 : C + 1],
                ps[:, 0 : 4 * (C + 1)].rearrange("p (a b) -> p a b", b=C + 1),
                0.0,
            )

        # ---- phase 2: attention + output conv, per query chunk ----
        # The normalize + output conv of chunk qc is emitted inside chunk
        # qc+1's loop so the PE FIFO is not blocked at chunk boundaries.
        def finalize(acc, q0, qn):
            # row C of acc = sum_k exp(score); reciprocal, then broadcast to
            # 64 partitions with a K=1 matmul against a ones column.
            recip1 = wk_pool.tile([1, 512], F32R, tag="recip1", name="recip1")
            with nc.allow_low_precision(reason="fp32r reciprocal, ~tf32 ok"):
                nc.vector.reciprocal(recip1[:, 0:qn], acc[C : C + 1, 0:qn])
            rb_ps = psA.tile([128, G, 512], F32, tag="sc", name="rb_ps")
            nc.tensor.matmul(rb_ps[0:C, 0, 0:qn], ones1[:], recip1[:, 0:qn])
            rbS = wk_pool.tile([C, 512], F32, tag="rbS", name="rbS")
            nc.vector.tensor_copy(rbS[:, 0:qn], rb_ps[0:C, 0, 0:qn])
            attn = wk_pool.tile([C, 512], F32R, tag="attn", name="attn")
            nc.vector.tensor_mul(attn[:, 0:qn], acc[0:C, 0:qn], rbS[:, 0:qn])
            # out = relu(wo_x @ x + wo_a @ attn + bo)
            fin = psO.tile([128, 512], F32, tag="acc", name="fin")
            nc.tensor.matmul(
                fin[0:C, 0:qn], WOX[:], XA[:, q0 : q0 + qn],
                start=True, stop=False,
            )
            nc.tensor.matmul(
                fin[0:C, 0:qn], WOA[:], attn[:, 0:qn],
                start=False, stop=True,
            )
            outs = wk_pool.tile([C, 512], F32, tag="outs", name="outs")
            nc.vector.tensor_scalar_max(outs[:, 0:qn], fin[0:C, 0:qn], 0.0)
            nc.sync.dma_start(out_d.ap()[:, q0 : q0 + qn], outs[:, 0:qn])

        # Flat software-pipelined emission over t = (chunk, j).  Each j emits
        # the mm1 trio and the two exp halves; DoubleRow mm2 pairs for a
        # completed super-step (2 j's) are drained with ~1-step lag so the PE
        # FIFO never waits on a just-issued exp.
        NT = len(QCHUNKS) * NJ
        accs = [None] * len(QCHUNKS)
        pend_pairs = []   # (qi, jj, p, st2_tile, qn)
        pend_fin = None
        vr_next = 0

        def emit_pair(qi, jj, p, st2, qn):
            kt = 6 * jj + 2 * p
            r, i = divmod(kt, 4)
            nc.tensor.matmul(
                accs[qi][0 : C + 1, 0:qn],
                VTr[r][:, i : i + 2, 0 : C + 1],
                st2[:, 2 * p : 2 * p + 2, 0:qn],
                start=(jj == 0 and p == 0),
                stop=(jj == NJJ - 1 and p == 2),
                perf_mode=mybir.MatmulPerfMode.DoubleRow,
            )

        st2_cur = None
        for t in range(NT):
            qi, j = divmod(t, NJ)
            jj = j // 2
            q0, qn = QCHUNKS[qi]
            if j == 0:
                accs[qi] = psO.tile([128, 512], F32, tag="acc", name="acc")
            if qi == 0:
                # remaining k-conv quarters, front-loaded so each quarter's
                # interleave DMA lands well before its first mm1 consumer
                # (quarter p needed from j = 6p)
                KSCHED = [3, 3, 2, 2, 2, 2, 2, 2]
                if j < len(KSCHED):
                    base = 6 + sum(KSCHED[:j])
                    for kc in range(base, base + KSCHED[j]):
                        k_chunk(kc // 6, kc % 6)
                # remaining q-conv chunks, well before their chunk starts
                if j in (2, 4, 6, 8):
                    q_chunk(j // 2)
            # keep v-conv a little ahead of the mm2 consumer on chunk 0
            while vr_next < NVR and (qi > 0 or 4 * vr_next <= 3 * j + 6):
                v_round(vr_next)
                vr_next += 1
            sc = psA.tile([128, G, 512], F32, tag="sc")
            jq, jjq = divmod(j, NJ // NQT)
            for g in range(G):
                nc.tensor.matmul(
                    sc[:, g, 0:qn],
                    KRq[jq][32 * g : 32 * g + 32, jjq, :],
                    QRc[qi][32 * g : 32 * g + 32, 0:qn],
                )
            if j % 2 == 0:
                st2_cur = stp.tile([128, 6, 512], F8, tag="st")
            g0 = 3 * (j % 2)
            rc = (R_SPLIT * qn) // 512
            nc.scalar.activation(
                st2_cur[:, g0 : g0 + 3, 0:rc],
                sc[:, :, 0:rc],
                mybir.ActivationFunctionType.Exp,
                bias=BIASM2[:],
            )
            nc.vector.tensor_scalar(
                st2_cur[:, g0 : g0 + 3, rc:qn].bitcast(U8),
                sc[:, :, rc:qn],
                float(SCHR_A), float(SCHR_B),
                mybir.AluOpType.mult, mybir.AluOpType.add,
            )
            if j % 2 == 1:
                for p in range(3):
                    pend_pairs.append((qi, jj, p, st2_cur, qn))
            # drain mm2 pairs with a ~1-step lag (do not emit pairs of the
            # super-step whose exp was just issued this t)
            budget = 2 if j % 2 == 0 else 1
            while pend_pairs and budget > 0:
                pqi, pjj, pp, pst2, pqn = pend_pairs[0]
                if pqi == qi and pjj == jj:
                    break
                emit_pair(*pend_pairs.pop(0))
                budget -= 1
            if j == 4 and pend_fin is not None:
                finalize(*pend_fin)
                pend_fin = None
            if j == NJ - 1:
                pend_fin = (accs[qi], q0, qn)
        for pair in pend_pairs:
            emit_pair(*pair)
        finalize(*pend_fin)

    nc.compile()
    return nc


_NC_CACHE = None


def _get_nc():
    global _NC_CACHE
    if _NC_CACHE is None:
        _NC_CACHE = build_bass()
    return _NC_CACHE


def make_in_maps(x, fusionmap, wq, bq, wk, bk, wa, ba, wo, bo):
    x = np.asarray(x, np.float32)
    fm = np.asarray(fusionmap, np.float32)
    xf = x.reshape(N, C, HW)
    fmf = fm.reshape(N, FC, HW)
    ones_hw = np.ones((1, HW), np.float32)
    wq_aug = np.concatenate(
        [np.asarray(wq).T, np.asarray(bq)[None, :]], 0
    ).astype(np.float32)
    wk_aug = np.concatenate(
        [np.asarray(wk).T, np.asarray(bk)[None, :]], 0
    ).astype(ml_dtypes.bfloat16)
    # [wa^T | 0; ba | 1]: columns C..2C-1 evaluate to exactly 1.0 after the
    # conv (ones row of fm_aug x ones), giving mm2 its denominator columns.
    wa_blk = np.concatenate([np.asarray(wa).T, np.asarray(ba)[None, :]], 0)
    ones_blk = np.concatenate(
        [np.zeros((FC, 1), np.float32), np.ones((1, 1), np.float32)], 0
    )
    wa_aug = np.concatenate([wa_blk, ones_blk], 1).astype(ml_dtypes.bfloat16)
    wo = np.asarray(wo, np.float32)
    wox_aug = np.concatenate(
        [wo[:, :C].T, np.asarray(bo)[None, :]], 0
    ).astype(np.float32)
    woa_t = np.ascontiguousarray(wo[:, C:].T).astype(np.float32)

    in_maps = []
    for core in range(NCORES):
        n, c = divmod(core, 4)
        x_chunk = xf[n][:, c * QPC : (c + 1) * QPC]
        x_aug = np.concatenate([x_chunk, ones_hw[:, :QPC]], 0)
        fm_aug = np.concatenate([fmf[n], ones_hw], 0).astype(ml_dtypes.bfloat16)
        in_maps.append(
            {
                "x_aug": np.ascontiguousarray(x_aug),
                "fm_aug": np.ascontiguousarray(fm_aug),
                "wq_aug": wq_aug,
                "wk_aug": wk_aug,
                "wa_aug": wa_aug,
                "wox_aug": wox_aug,
                "woa_t": woa_t,
            }
        )
    return in_maps


def run(in_maps, trace=False, tmpdir=None):
    nc = _get_nc()
    return bass_utils.run_bass_kernel_spmd(
        nc,
        in_maps,
        core_ids=list(range(NCORES)),
        trace=trace,
        tmpdir=tmpdir,
    )


def kernel(**inputs):
    in_maps = make_in_maps(**inputs)
    res = run(in_maps)
    out = np.empty((N, C, HW), np.float32)
    for core in range(NCORES):
        n, c = divmod(core, 4)
        out[n][:, c * QPC : (c + 1) * QPC] = res.results[core]["out_c"]
    return out.reshape(N, C, H, W)


if __name__ == "__main__":
    import reference

    inputs = {k: np.asarray(v) for k, v in reference.setup_inputs().items()}
    got = kernel(**inputs)
    print("kernel output", got.shape, got.dtype)


# revision 8
# speedup vs baseline: 1.0363x; 1.0309x over previous
"""Trainium2 Bass kernel for NonLocalAttention (fused 1x1 convs + spatial softmax attention).

Reference computation (N=2, C=64, FC=64, CR=32, H=W=96, HW=9216):
    q = relu(wq @ x + bq)          [N, 32, HW]
    k = relu(wk @ fm + bk)         [N, 32, HW]
    v = relu(wa @ fm + ba)         [N, 64, HW]
    s = softmax(q^T k, axis=keys)  [N, HW, HW]
    o = s @ v^T                    [N, HW, 64]
    out = relu(wo @ [x; o^T] + bo) [N, 64, HW]

Sharding: 8 cores = batch(2) x query-rows(4).  Each core handles 2304 query
pixels of one batch element and needs the full fusionmap of that batch.

Per-core kernel (flash-style, score never goes to HBM):
  - score is computed TRANSPOSED: st[key, q] = k^T q via row-packed (K=32)
    bf16 matmuls, 3 key-tiles of 128 at a time into 3 PSUM banks (the three
    row-group matmuls run concurrently in the PE array).
  - scores are >= 0 (q and k are relu'd), bounded ~6.5, so softmax needs no
    max subtraction.  exp uses a fixed bias of -2 so exp(s-2) in [0.14, ~90]
    fits fp8e4m3 comfortably (max finite 240); the constant factor cancels in
    the softmax ratio.
  - exp is split between BOTH elementwise engines to halve the exp wall time
    (it is the kernel bottleneck at ~170us if done on ScalarE alone):
      * ScalarE: true exp activation, PSUM -> fp8e4 SBUF, query cols [0, R)
      * VectorE: Schraudolph fast exp - one tensor_scalar (x*A + B) with
        round-to-nearest conversion to uint8; the byte IS the fp8e4m3 bit
        pattern of 2^((i-56)/8) ~= exp(x-2).  Query cols [R, qn).
    The split is by query column, so each query's full softmax row (numerator
    and denominator) uses one engine consistently and the per-engine
    multiplicative error largely cancels in the ratio.
  - mm2 contracts keys with fp8 DoubleRow matmuls: each matmul processes TWO
    key tiles (virtual 128x256 array) with lhsT = [vT | 1] pairs, so PSUM row
    64 accumulates the softmax denominator for free and mm2 PE time halves.
  - normalize with reciprocal + DMA partition-broadcast, then the output
    1x1 conv (wo) is fused in, relu, DMA out.
"""

import sys

sys.path.insert(0, "/opt/trn_rl_repo")

from contextlib import ExitStack

import ml_dtypes
import numpy as np

import concourse.bacc as bacc
import concourse.bass as bass
import concourse.tile as tile
from concourse import mybir
from concourse import bass_utils

C = 64
FC = 64
CR = 32
N = 2
H = W = 96
HW = H * W            # 9216
NCORES = 8
QPC = HW // 4         # queries per core = 2304
NKT = HW // 128       # 72 key tiles
G = 3                 # row-packing group (3 key tiles concurrently)
NJ = NKT // G         # 24 key-tile groups
NJJ = NJ // 2         # 12 super-steps (2 j's = 6 key tiles = 3 DoubleRow pairs)
QCHUNKS = [(0, 512), (512, 512), (1024, 512), (1536, 512), (2048, 256)]

F32 = mybir.dt.float32
F32R = mybir.dt.float32r
BF16 = mybir.dt.bfloat16
F8 = mybir.dt.float8e4
U8 = mybir.dt.uint8
ATT = BF16            # q/k dtype for mm1

# exp split point: ScalarE does query cols [0, R) of each 512-wide chunk,
# VectorE (Schraudolph) does [R, 512).  Scaled proportionally for the 256
# chunk.  Tuned so both engines' per-step time balances.
R_SPLIT = 320

# Schraudolph constants: byte = round(A*s + B) is the fp8e4m3 bit pattern of
# ~exp(s-2).  c = -0.42 centers the log-ratio error (spread ~0.146, on par
# with plain e4m3 quantization of exact exp at 0.118).
SCHR_A = 8.0 / np.log(2.0)
SCHR_B = 56.0 - 2.0 * SCHR_A - 0.42


def build_bass():
    nc = bacc.Bacc(
        "TRN2", target_bir_lowering=False, debug=False, num_devices=NCORES
    )

    x_aug = nc.dram_tensor("x_aug", [C + 1, QPC], F32R, kind="ExternalInput")
    fm_aug = nc.dram_tensor("fm_aug", [FC + 1, HW], BF16, kind="ExternalInput")
    wq_aug = nc.dram_tensor("wq_aug", [C + 1, CR], F32R, kind="ExternalInput")
    wk_aug = nc.dram_tensor("wk_aug", [FC + 1, CR], BF16, kind="ExternalInput")
    wa_aug = nc.dram_tensor("wa_aug", [FC + 1, C + 1], BF16, kind="ExternalInput")
    wox_aug = nc.dram_tensor("wox_aug", [C + 1, C], F32R, kind="ExternalInput")
    woa_t = nc.dram_tensor("woa_t", [C, C], F32R, kind="ExternalInput")
    out_d = nc.dram_tensor("out_c", [C, QPC], F32, kind="ExternalOutput")

    with tile.TileContext(nc) as tc, ExitStack() as ctx:
        consts = ctx.enter_context(tc.tile_pool(name="consts", bufs=1))
        stp = ctx.enter_context(tc.tile_pool(name="stp", bufs=2))
        wk_pool = ctx.enter_context(tc.tile_pool(name="work", bufs=3))
        # PSUM: 2x3 banks score tiles + 2x1 bank shared acc/fin/v-conv = 8
        psA = ctx.enter_context(tc.tile_pool(name="psA", bufs=2, space="PSUM"))
        psO = ctx.enter_context(tc.tile_pool(name="psO", bufs=2, space="PSUM"))

        # ---- constants / inputs in SBUF ----
        # fusionmap in 4 quarters so k-conv starts before the full DMA lands
        NQT = 4
        HWQ = HW // NQT  # 2304 = 18 key tiles per quarter
        FMq = [
            consts.tile([FC + 1, HWQ], BF16, tag=f"fm{p}", name=f"FM{p}")
            for p in range(NQT)
        ]
        XA = consts.tile([C + 1, QPC], F32R)         # x chunk + ones row
        WQ = consts.tile([C + 1, CR], F32R)
        WK = consts.tile([FC + 1, CR], BF16)
        WA = consts.tile([FC + 1, C + 1], BF16)
        WOX = consts.tile([C + 1, C], F32R)
        WOA = consts.tile([C, C], F32R)
        # DMA order tuned for earliest compute start: k-conv needs WK + the
        # first FM0 piece; q-conv needs WQ + the first XA piece.  Quarters
        # are split into pieces so subtile deps let convs start before the
        # whole quarter lands.
        nc.sync.dma_start(WK[:], wk_aug.ap())
        nc.sync.dma_start(WQ[:], wq_aug.ap())
        nc.sync.dma_start(WA[:], wa_aug.ap())
        for s in range(3):
            nc.sync.dma_start(
                FMq[0][:, 768 * s : 768 * (s + 1)],
                fm_aug.ap()[:, 768 * s : 768 * (s + 1)],
            )
        nc.sync.dma_start(XA[:, 0:512], x_aug.ap()[:, 0:512])
        for p in range(1, NQT):
            nc.sync.dma_start(FMq[p][:], fm_aug.ap()[:, p * HWQ : (p + 1) * HWQ])
        nc.sync.dma_start(XA[:, 512:QPC], x_aug.ap()[:, 512:QPC])
        nc.sync.dma_start(WOX[:], wox_aug.ap())
        nc.sync.dma_start(WOA[:], woa_t.ap())

        def fm_kt(kt):  # [65, 128] slice of fusionmap for key tile kt
            p, i = divmod(kt, 18)
            return FMq[p][:, 128 * i : 128 * (i + 1)]

        # KR: k channels row-packed: partitions 32g..32g+31 hold key tile
        # kt=3j+g at free block j; split in j-quarters for earlier start.
        # QR: per-chunk tiles, q replicated on partition groups 0..2.
        KRq = [
            consts.tile([128, NJ // NQT, 128], ATT, tag=f"kr{p}", name=f"KR{p}")
            for p in range(NQT)
        ]
        QRc = [
            consts.tile([128, qn], ATT, tag=f"qr{ci}", name=f"QR{ci}")
            for ci, (q0, qn) in enumerate(QCHUNKS)
        ]
        # VT: [keys(128), kt, C+1] in fp8e4, padded to 80 so a DoubleRow
        # weight pair AP [128, 2, 65] has a 16-byte-multiple pair stride.
        # Column C is 1.0 straight out of the v-conv (wa is augmented with a
        # ones column), so mm2 accumulates the softmax denominator in PSUM
        # row C for free.
        NVR = NKT // 4
        VTr = [
            consts.tile([128, 4, 80], F8, tag=f"vt{r}", name=f"VT{r}")
            for r in range(NVR)
        ]
        ones1 = consts.tile([1, C], F32R)
        nc.vector.memset(ones1[:].bitcast(F32), 1.0)
        BIASM2 = consts.tile([128, 1], F32)
        nc.vector.memset(BIASM2[:], -2.0)

        # Preload the exp table set (~2.7us) off the critical path, before
        # the first real exp.
        dummy = wk_pool.tile([1, 1], F32, tag="dummy", name="dummy")
        nc.scalar.activation(
            dummy[:], ones1[0:1, 0:1].bitcast(F32),
            mybir.ActivationFunctionType.Exp,
        )

        # PE warm-up: back-to-back dummy matmuls on a zero tile.  The PE HAM
        # clock gate defaults to 1.2 GHz and only un-throttles after a
        # ~3.4us continuously-busy window (and re-throttles after a ~3.4us
        # idle window); without this the whole kernel runs at half PE clock.
        # warm(n) is also trickled between the DMA-gated head convs so the
        # PE never sees a long idle gap before the attention loop sustains
        # it.
        warm_src = consts.tile([128, 512], BF16, name="warm_src")
        nc.gpsimd.memset(warm_src[:], 0.0)

        def warm(n):
            ps = psA.tile([128, G, 512], F32, tag="sc", name="warm_ps")
            for _ in range(n):
                nc.tensor.matmul(ps[:, 0, 0:512], warm_src[:, 0:128], warm_src[:])

        warm(14)

        # ---- phase 1: q / k convs (emitted interleaved with phase 2) ----
        # fp32r matmuls must write PSUM partition 0, so produce plain q/k at
        # partitions 0-31 and rearrange into the packed layouts via DMA.
        # k quarter p: plain [32, HW/4] as 6 x 384-wide chunks, then
        # interleave kt%3 -> partition group via DMA
        Ksq = [
            consts.tile([CR, HWQ], ATT, tag=f"ks{p}", name=f"Ks{p}")
            for p in range(NQT)
        ]

        def k_chunk(p, c):
            ps = psA.tile([128, G, 512], F32, tag="sc", name="kps")
            nc.tensor.matmul(
                ps[0:CR, 0, 0:384], WK[:], FMq[p][:, 384 * c : 384 * (c + 1)]
            )
            nc.vector.tensor_scalar_max(
                Ksq[p][:, 384 * c : 384 * (c + 1)], ps[0:CR, 0, 0:384], 0.0
            )
            if c == HWQ // 384 - 1:
                Ksv = Ksq[p].rearrange("p (j g c) -> p j g c", g=G, c=128)
                for g in range(G):
                    nc.sync.dma_start(
                        KRq[p][32 * g : 32 * g + 32, :, :], Ksv[:, :, g, :]
                    )

        # q chunk ci: relu into QRc[ci][0:32], then replicate to groups 1, 2
        def q_chunk(ci):
            q0, qn = QCHUNKS[ci]
            ps = psA.tile([128, G, 512], F32, tag="sc", name="qps")
            nc.tensor.matmul(
                ps[0:CR, 0, 0:qn], WQ[:], XA[:, q0 : q0 + qn]
            )
            nc.vector.tensor_scalar_max(
                QRc[ci][0:CR, 0:qn], ps[0:CR, 0, 0:qn], 0.0
            )
            nc.sync.dma_start(QRc[ci][32:64, 0:qn], QRc[ci][0:32, 0:qn])
            nc.sync.dma_start(QRc[ci][64:96, 0:qn], QRc[ci][0:32, 0:qn])

        # head: only quarter-0 k-conv + chunk-0 q-conv precede the main
        # loop (the rest are emitted inside chunk 0's attention loop via
        # KSCHED / the j-schedule).  Dummy warm matmuls between them absorb
        # the DMA waits so the HAM clock gate stays un-throttled.
        for c in range(HWQ // 384):
            k_chunk(0, c)
            warm(2)
        q_chunk(0)
        warm(6)

        # v^T conv round r: out[key, 0:64] = relu'd v, out[key, 64] = 1.0
        # (from the wa ones block) for key tiles 4r..4r+3 into one PSUM bank,
        # relu'd + converted to fp8 in one DVE op into the padded VT layout.
        def v_round(r):
            ps = psO.tile([128, 512], F32, tag="acc")
            for i in range(4):
                kt = 4 * r + i
                nc.tensor.matmul(
                    ps[:, (C + 1) * i : (C + 1) * (i + 1)], fm_kt(kt), WA[:]
                )
            nc.vector.tensor_scalar_max(
                VTr[r][:, 0:4, 0 : C + 1],
                ps[:, 0 : 4 * (C + 1)].rearrange("p (a b) -> p a b", b=C + 1),
                0.0,
            )

        # ---- phase 2: attention + output conv, per query chunk ----
        # The normalize + output conv of chunk qc is emitted inside chunk
        # qc+1's loop so the PE FIFO is not blocked at chunk boundaries.
        def finalize(acc, q0, qn):
            # row C of acc = sum_k exp(score); reciprocal, then broadcast to
            # 64 partitions with a K=1 matmul against a ones column.
            recip1 = wk_pool.tile([1, 512], F32R, tag="recip1", name="recip1")
            with nc.allow_low_precision(reason="fp32r reciprocal, ~tf32 ok"):
                nc.vector.reciprocal(recip1[:, 0:qn], acc[C : C + 1, 0:qn])
            rb_ps = psA.tile([128, G, 512], F32, tag="sc", name="rb_ps")
            nc.tensor.matmul(rb_ps[0:C, 0, 0:qn], ones1[:], recip1[:, 0:qn])
            rbS = wk_pool.tile([C, 512], F32, tag="rbS", name="rbS")
            nc.vector.tensor_copy(rbS[:, 0:qn], rb_ps[0:C, 0, 0:qn])
            attn = wk_pool.tile([C, 512], F32R, tag="attn", name="attn")
            nc.vector.tensor_mul(attn[:, 0:qn], acc[0:C, 0:qn], rbS[:, 0:qn])
            # out = relu(wo_x @ x + wo_a @ attn + bo)
            fin = psO.tile([128, 512], F32, tag="acc", name="fin")
            nc.tensor.matmul(
                fin[0:C, 0:qn], WOX[:], XA[:, q0 : q0 + qn],
                start=True, stop=False,
            )
            nc.tensor.matmul(
                fin[0:C, 0:qn], WOA[:], attn[:, 0:qn],
                start=False, stop=True,
            )
            outs = wk_pool.tile([C, 512], F32, tag="outs", name="outs")
            nc.vector.tensor_scalar_max(outs[:, 0:qn], fin[0:C, 0:qn], 0.0)
            nc.sync.dma_start(out_d.ap()[:, q0 : q0 + qn], outs[:, 0:qn])

        # Flat software-pipelined emission over t = (chunk, j).  Each j emits
        # the mm1 trio and the two exp halves; DoubleRow mm2 pairs for a
        # completed super-step (2 j's) are drained with ~1-step lag so the PE
        # FIFO never waits on a just-issued exp.
        NT = len(QCHUNKS) * NJ
        accs = [None] * len(QCHUNKS)
        pend_pairs = []   # (qi, jj, p, st2_tile, qn)
        pend_fin = None
        vr_next = 0

        def emit_pair(qi, jj, p, st2, qn):
            kt = 6 * jj + 2 * p
            r, i = divmod(kt, 4)
            nc.tensor.matmul(
                accs[qi][0 : C + 1, 0:qn],
                VTr[r][:, i : i + 2, 0 : C + 1],
                st2[:, 2 * p : 2 * p + 2, 0:qn],
                start=(jj == 0 and p == 0),
                stop=(jj == NJJ - 1 and p == 2),
                perf_mode=mybir.MatmulPerfMode.DoubleRow,
            )

        st2_cur = None
        for t in range(NT):
            qi, j = divmod(t, NJ)
            jj = j // 2
            q0, qn = QCHUNKS[qi]
            if j == 0:
                accs[qi] = psO.tile([128, 512], F32, tag="acc", name="acc")
            if qi == 0:
                # remaining k-conv quarters, front-loaded so each quarter's
                # interleave DMA lands well before its first mm1 consumer
                # (quarter p needed from j = 6p)
                KSCHED = [3, 3, 2, 2, 2, 2, 2, 2]
                if j < len(KSCHED):
                    base = 6 + sum(KSCHED[:j])
                    for kc in range(base, base + KSCHED[j]):
                        k_chunk(kc // 6, kc % 6)
                # remaining q-conv chunks, well before their chunk starts
                if j in (2, 4, 6, 8):
                    q_chunk(j // 2)
            # keep v-conv a little ahead of the mm2 consumer on chunk 0
            while vr_next < NVR and (qi > 0 or 4 * vr_next <= 3 * j + 6):
                v_round(vr_next)
                vr_next += 1
            sc = psA.tile([128, G, 512], F32, tag="sc")
            jq, jjq = divmod(j, NJ // NQT)
            for g in range(G):
                nc.tensor.matmul(
                    sc[:, g, 0:qn],
                    KRq[jq][32 * g : 32 * g + 32, jjq, :],
                    QRc[qi][32 * g : 32 * g + 32, 0:qn],
                )
            if j % 2 == 0:
                st2_cur = stp.tile([128, 6, 512], F8, tag="st")
            g0 = 3 * (j % 2)
            rc = (R_SPLIT * qn) // 512
            nc.scalar.activation(
                st2_cur[:, g0 : g0 + 3, 0:rc],
                sc[:, :, 0:rc],
                mybir.ActivationFunctionType.Exp,
                bias=BIASM2[:],
            )
            nc.vector.tensor_scalar(
                st2_cur[:, g0 : g0 + 3, rc:qn].bitcast(U8),
                sc[:, :, rc:qn],
                float(SCHR_A), float(SCHR_B),
                mybir.AluOpType.mult, mybir.AluOpType.add,
            )
            if j % 2 == 1:
                for p in range(3):
                    pend_pairs.append((qi, jj, p, st2_cur, qn))
            # drain mm2 pairs with a ~1-step lag (do not emit pairs of the
            # super-step whose exp was just issued this t)
            budget = 2 if j % 2 == 0 else 1
            while pend_pairs and budget > 0:
                pqi, pjj, pp, pst2, pqn = pend_pairs[0]
                if pqi == qi and pjj == jj:
                    break
                emit_pair(*pend_pairs.pop(0))
                budget -= 1
            if j == 4 and pend_fin is not None:
                finalize(*pend_fin)
                pend_fin = None
            if j == NJ - 1:
                pend_fin = (accs[qi], q0, qn)
        for pair in pend_pairs:
            emit_pair(*pair)
        finalize(*pend_fin)

    nc.compile()
    return nc


_NC_CACHE = None


def _get_nc():
    global _NC_CACHE
    if _NC_CACHE is None:
        _NC_CACHE = build_bass()
    return _NC_CACHE


def make_in_maps(x, fusionmap, wq, bq, wk, bk, wa, ba, wo, bo):
    x = np.asarray(x, np.float32)
    fm = np.asarray(fusionmap, np.float32)
    xf = x.reshape(N, C, HW)
    fmf = fm.reshape(N, FC, HW)
    ones_hw = np.ones((1, HW), np.float32)
    wq_aug = np.concatenate(
        [np.asarray(wq).T, np.asarray(bq)[None, :]], 0
    ).astype(np.float32)
    wk_aug = np.concatenate(
        [np.asarray(wk).T, np.asarray(bk)[None, :]], 0
    ).astype(ml_dtypes.bfloat16)
    # [wa^T | 0; ba | 1]: columns C..2C-1 evaluate to exactly 1.0 after the
    # conv (ones row of fm_aug x ones), giving mm2 its denominator columns.
    wa_blk = np.concatenate([np.asarray(wa).T, np.asarray(ba)[None, :]], 0)
    ones_blk = np.concatenate(
        [np.zeros((FC, 1), np.float32), np.ones((1, 1), np.float32)], 0
    )
    wa_aug = np.concatenate([wa_blk, ones_blk], 1).astype(ml_dtypes.bfloat16)
    wo = np.asarray(wo, np.float32)
    wox_aug = np.concatenate(
        [wo[:, :C].T, np.asarray(bo)[None, :]], 0
    ).astype(np.float32)
    woa_t = np.ascontiguousarray(wo[:, C:].T).astype(np.float32)

    in_maps = []
    for core in range(NCORES):
        n, c = divmod(core, 4)
        x_chunk = xf[n][:, c * QPC : (c + 1) * QPC]
        x_aug = np.concatenate([x_chunk, ones_hw[:, :QPC]], 0)
        fm_aug = np.concatenate([fmf[n], ones_hw], 0).astype(ml_dtypes.bfloat16)
        in_maps.append(
            {
                "x_aug": np.ascontiguousarray(x_aug),
                "fm_aug": np.ascontiguousarray(fm_aug),
                "wq_aug": wq_aug,
                "wk_aug": wk_aug,
                "wa_aug": wa_aug,
                "wox_aug": wox_aug,
                "woa_t": woa_t,
            }
        )
    return in_maps


def run(in_maps, trace=False, tmpdir=None):
    nc = _get_nc()
    return bass_utils.run_bass_kernel_spmd(
        nc,
        in_maps,
        core_ids=list(range(NCORES)),
        trace=trace,
        tmpdir=tmpdir,
    )


def kernel(**inputs):
    in_maps = make_in_maps(**inputs)
    res = run(in_maps)
    out = np.empty((N, C, HW), np.float32)
    for core in range(NCORES):
        n, c = divmod(core, 4)
        out[n][:, c * QPC : (c + 1) * QPC] = res.results[core]["out_c"]
    return out.reshape(N, C, H, W)


if __name__ == "__main__":
    import reference

    inputs = {k: np.asarray(v) for k, v in reference.setup_inputs().items()}
    got = kernel(**inputs)
    print("kernel output", got.shape, got.dtype)


# revision 12
# speedup vs baseline: 1.0556x; 1.0186x over previous
"""Trainium2 Bass kernel for NonLocalAttention (fused 1x1 convs + spatial softmax attention).

Reference computation (N=2, C=64, FC=64, CR=32, H=W=96, HW=9216):
    q = relu(wq @ x + bq)          [N, 32, HW]
    k = relu(wk @ fm + bk)         [N, 32, HW]
    v = relu(wa @ fm + ba)         [N, 64, HW]
    s = softmax(q^T k, axis=keys)  [N, HW, HW]
    o = s @ v^T                    [N, HW, 64]
    out = relu(wo @ [x; o^T] + bo) [N, 64, HW]

Sharding: 8 cores = batch(2) x query-rows(4).  Each core handles 2304 query
pixels of one batch element and needs the full fusionmap of that batch.

Per-core kernel (flash-style, score never goes to HBM):
  - score is computed TRANSPOSED: st[key, q] = k^T q via row-packed (K=32)
    bf16 matmuls, 3 key-tiles of 128 at a time into 3 PSUM banks (the three
    row-group matmuls run concurrently in the PE array).
  - scores are >= 0 (q and k are relu'd), bounded ~6.5, so softmax needs no
    max subtraction.  exp uses a fixed bias of -2 so exp(s-2) in [0.14, ~90]
    fits fp8e4m3 comfortably (max finite 240); the constant factor cancels in
    the softmax ratio.
  - exp is split between BOTH elementwise engines to halve the exp wall time
    (it is the kernel bottleneck at ~170us if done on ScalarE alone):
      * ScalarE: true exp activation, PSUM -> fp8e4 SBUF, query cols [0, R)
      * VectorE: Schraudolph fast exp - one tensor_scalar (x*A + B) with
        round-to-nearest conversion to uint8; the byte IS the fp8e4m3 bit
        pattern of 2^((i-56)/8) ~= exp(x-2).  Query cols [R, qn).
    The split is by query column, so each query's full softmax row (numerator
    and denominator) uses one engine consistently and the per-engine
    multiplicative error largely cancels in the ratio.
  - mm2 contracts keys with fp8 DoubleRow matmuls: each matmul processes TWO
    key tiles (virtual 128x256 array) with lhsT = [vT | 1] pairs, so PSUM row
    64 accumulates the softmax denominator for free and mm2 PE time halves.
  - normalize with reciprocal + DMA partition-broadcast, then the output
    1x1 conv (wo) is fused in, relu, DMA out.
"""

import sys

sys.path.insert(0, "/opt/trn_rl_repo")

from contextlib import ExitStack

import ml_dtypes
import numpy as np

import concourse.bacc as bacc
import concourse.bass as bass
import concourse.tile as tile
from concourse import mybir
from concourse import bass_utils

C = 64
FC = 64
CR = 32
N = 2
H = W = 96
HW = H * W            # 9216
NCORES = 8
QPC = HW // 4         # queries per core = 2304
NKT = HW // 128       # 72 key tiles
G = 3                 # row-packing group (3 key tiles concurrently)
NJ = NKT // G         # 24 key-tile groups
NJJ = NJ // 2         # 12 super-steps (2 j's = 6 key tiles = 3 DoubleRow pairs)
QCHUNKS = [(0, 512), (512, 512), (1024, 512), (1536, 512), (2048, 256)]

F32 = mybir.dt.float32
F32R = mybir.dt.float32r
BF16 = mybir.dt.bfloat16
F8 = mybir.dt.float8e4
U8 = mybir.dt.uint8
ATT = BF16            # q/k dtype for mm1

# exp split point: ScalarE does query cols [0, R) of each 512-wide chunk,
# VectorE (Schraudolph) does [R, 512).  Scaled proportionally for the 256
# chunk.  Tuned so both engines' per-step time balances.
R_SPLIT = 320

# Schraudolph constants: byte = round(A*s + B) is the fp8e4m3 bit pattern of
# ~exp(s-2).  c = -0.42 centers the log-ratio error (spread ~0.146, on par
# with plain e4m3 quantization of exact exp at 0.118).
SCHR_A = 8.0 / np.log(2.0)
SCHR_B = 56.0 - 2.0 * SCHR_A - 0.42


def build_bass():
    nc = bacc.Bacc(
        "TRN2", target_bir_lowering=False, debug=False, num_devices=NCORES
    )

    x_aug = nc.dram_tensor("x_aug", [C + 1, QPC], F32R, kind="ExternalInput")
    fm_aug = nc.dram_tensor("fm_aug", [FC + 1, HW], BF16, kind="ExternalInput")
    wq_aug = nc.dram_tensor("wq_aug", [C + 1, CR], F32R, kind="ExternalInput")
    wk_aug = nc.dram_tensor("wk_aug", [FC + 1, CR], BF16, kind="ExternalInput")
    wa_aug = nc.dram_tensor("wa_aug", [FC + 1, C + 1], BF16, kind="ExternalInput")
    wox_aug = nc.dram_tensor("wox_aug", [C + 1, C], F32R, kind="ExternalInput")
    woa_t = nc.dram_tensor("woa_t", [C, C], F32R, kind="ExternalInput")
    out_d = nc.dram_tensor("out_c", [C, QPC], F32, kind="ExternalOutput")

    with tile.TileContext(nc) as tc, ExitStack() as ctx:
        consts = ctx.enter_context(tc.tile_pool(name="consts", bufs=1))
        stp = ctx.enter_context(tc.tile_pool(name="stp", bufs=2))
        wk_pool = ctx.enter_context(tc.tile_pool(name="work", bufs=3))
        # PSUM: 2x3 banks score tiles + 2x1 bank shared acc/fin/v-conv = 8
        psA = ctx.enter_context(tc.tile_pool(name="psA", bufs=2, space="PSUM"))
        psO = ctx.enter_context(tc.tile_pool(name="psO", bufs=2, space="PSUM"))

        # ---- constants / inputs in SBUF ----
        # fusionmap in 4 quarters so k-conv starts before the full DMA lands
        NQT = 4
        HWQ = HW // NQT  # 2304 = 18 key tiles per quarter
        FMq = [
            consts.tile([FC + 1, HWQ], BF16, tag=f"fm{p}", name=f"FM{p}")
            for p in range(NQT)
        ]
        XA = consts.tile([C + 1, QPC], F32R)         # x chunk + ones row
        WQ = consts.tile([C + 1, CR], F32R)
        WK = consts.tile([FC + 1, CR], BF16)
        WA = consts.tile([FC + 1, C + 1], BF16)
        WOX = consts.tile([C + 1, C], F32R)
        WOA = consts.tile([C, C], F32R)
        # DMA order tuned for earliest compute start: k-conv needs WK + the
        # first FM0 piece; q-conv needs WQ + the first XA piece.  Quarters
        # are split into pieces so subtile deps let convs start before the
        # whole quarter lands.  The bulk transfers (FM1-3, XA tail) go out
        # on the Activation and GpSimd DMA queues, which are idle in the
        # head, so the sync queue (QR replication etc.) is not blocked
        # behind ~1MB of input traffic.
        nc.sync.dma_start(WK[:], wk_aug.ap())
        nc.sync.dma_start(WQ[:], wq_aug.ap())
        nc.sync.dma_start(WA[:], wa_aug.ap())
        for s in range(3):
            nc.sync.dma_start(
                FMq[0][:, 768 * s : 768 * (s + 1)],
                fm_aug.ap()[:, 768 * s : 768 * (s + 1)],
            )
        nc.sync.dma_start(XA[:, 0:512], x_aug.ap()[:, 0:512])
        for p in range(1, NQT):
            nc.scalar.dma_start(FMq[p][:], fm_aug.ap()[:, p * HWQ : (p + 1) * HWQ])
        nc.gpsimd.dma_start(XA[:, 512:QPC], x_aug.ap()[:, 512:QPC])
        nc.gpsimd.dma_start(WOX[:], wox_aug.ap())
        nc.gpsimd.dma_start(WOA[:], woa_t.ap())

        def fm_kt(kt):  # [65, 128] slice of fusionmap for key tile kt
            p, i = divmod(kt, 18)
            return FMq[p][:, 128 * i : 128 * (i + 1)]

        # KR: k channels row-packed: partitions 32g..32g+31 hold key tile
        # kt=3j+g at free block j; split in j-quarters for earlier start.
        # QR: per-chunk tiles, q replicated on partition groups 0..2.
        KRq = [
            consts.tile([128, NJ // NQT, 128], ATT, tag=f"kr{p}", name=f"KR{p}")
            for p in range(NQT)
        ]
        QRc = [
            consts.tile([128, qn], ATT, tag=f"qr{ci}", name=f"QR{ci}")
            for ci, (q0, qn) in enumerate(QCHUNKS)
        ]
        # VT: [keys(128), kt, C+1] in fp8e4, padded to 80 so a DoubleRow
        # weight pair AP [128, 2, 65] has a 16-byte-multiple pair stride.
        # Column C is 1.0 straight out of the v-conv (wa is augmented with a
        # ones column), so mm2 accumulates the softmax denominator in PSUM
        # row C for free.
        NVR = NKT // 4
        VTr = [
            consts.tile([128, 4, 80], F8, tag=f"vt{r}", name=f"VT{r}")
            for r in range(NVR)
        ]
        ones1 = consts.tile([1, C], F32R)
        nc.vector.memset(ones1[:].bitcast(F32), 1.0)
        BIASM2 = consts.tile([128, 1], F32)
        nc.vector.memset(BIASM2[:], -2.0)

        # Preload the exp table set (~2.7us) off the critical path, before
        # the first real exp.
        dummy = wk_pool.tile([1, 1], F32, tag="dummy", name="dummy")
        nc.scalar.activation(
            dummy[:], ones1[0:1, 0:1].bitcast(F32),
            mybir.ActivationFunctionType.Exp,
        )

        # PE warm-up: back-to-back dummy matmuls on a zero tile.  The PE HAM
        # clock gate defaults to 1.2 GHz and only un-throttles after a
        # ~3.4us continuously-busy window (and re-throttles after a ~3.4us
        # idle window); without this the whole kernel runs at half PE clock.
        # warm(n) is also trickled between the DMA-gated head convs so the
        # PE never sees a long idle gap before the attention loop sustains
        # it.
        warm_src = consts.tile([128, 512], BF16, name="warm_src")
        nc.gpsimd.memset(warm_src[:], 0.0)

        def warm(n):
            ps = psA.tile([128, G, 512], F32, tag="sc", name="warm_ps")
            for _ in range(n):
                nc.tensor.matmul(ps[:, 0, 0:512], warm_src[:, 0:128], warm_src[:])

        warm(14)

        # ---- phase 1: q / k convs (emitted interleaved with phase 2) ----
        # k-conv writes DIRECTLY into the row-packed KR layout: per key tile
        # kt = 3j+g, a col-group-targeted bf16 matmul puts k^T[kt] at PSUM
        # partitions 32g..32g+31 (the three col-groups run concurrently in
        # the PE), and one batched relu [96, 384] per 3-j block moves all 9
        # tiles to KR.  No Ksq staging, no interleave DMAs.
        def k_block(p, B):
            # quarter p, block B in {0, 1}: js 6p+3B .. +2, kts 18p+9B .. +8
            ps = psA.tile([128, G, 512], F32, tag="sc", name="kps")
            for cj in range(3):
                for g in range(G):
                    kt = 18 * p + 9 * B + 3 * cj + g
                    nc.tensor.matmul(
                        ps[32 * g : 32 * g + 32, 0, 128 * cj : 128 * (cj + 1)],
                        WK[:],
                        fm_kt(kt),
                    )
            jq0 = 3 * B
            nc.vector.tensor_scalar_max(
                KRq[p][0:96, jq0 : jq0 + 3, :],
                ps[0:96, 0, 0:384].rearrange("p (a b) -> p a b", b=128),
                0.0,
            )

        # q chunk ci: relu into QRc[ci][0:32], then replicate to groups 1, 2
        def q_chunk(ci):
            q0, qn = QCHUNKS[ci]
            ps = psA.tile([128, G, 512], F32, tag="sc", name="qps")
            nc.tensor.matmul(
                ps[0:CR, 0, 0:qn], WQ[:], XA[:, q0 : q0 + qn]
            )
            nc.vector.tensor_scalar_max(
                QRc[ci][0:CR, 0:qn], ps[0:CR, 0, 0:qn], 0.0
            )
            nc.sync.dma_start(QRc[ci][32:64, 0:qn], QRc[ci][0:32, 0:qn])
            nc.sync.dma_start(QRc[ci][64:96, 0:qn], QRc[ci][0:32, 0:qn])

        # head: only quarter-0 k-conv + chunk-0 q-conv precede the main
        # loop (the rest are emitted inside chunk 0's attention loop via
        # the j-schedule).  Dummy warm matmuls between them absorb the DMA
        # waits so the HAM clock gate stays un-throttled.
        q_chunk(0)
        warm(2)
        k_block(0, 0)
        warm(2)
        k_block(0, 1)
        warm(8)

        # v^T conv round r: out[key, 0:64] = relu'd v, out[key, 64] = 1.0
        # (from the wa ones block) for key tiles 4r..4r+3 into one PSUM bank,
        # relu'd + converted to fp8 in one DVE op into the padded VT layout.
        def v_round(r):
            ps = psO.tile([128, 512], F32, tag="acc")
            for i in range(4):
                kt = 4 * r + i
                nc.tensor.matmul(
                    ps[:, (C + 1) * i : (C + 1) * (i + 1)], fm_kt(kt), WA[:]
                )
            nc.vector.tensor_scalar_max(
                VTr[r][:, 0:4, 0 : C + 1],
                ps[:, 0 : 4 * (C + 1)].rearrange("p (a b) -> p a b", b=C + 1),
                0.0,
            )

        # ---- phase 2: attention + output conv, per query chunk ----
        # The normalize + output conv of chunk qc is emitted inside chunk
        # qc+1's loop so the PE FIFO is not blocked at chunk boundaries.
        def finalize(acc, q0, qn):
            # row C of acc = sum_k exp(score); reciprocal, then broadcast to
            # 64 partitions with a K=1 matmul against a ones column.
            recip1 = wk_pool.tile([1, 512], F32R, tag="recip1", name="recip1")
            with nc.allow_low_precision(reason="fp32r reciprocal, ~tf32 ok"):
                nc.vector.reciprocal(recip1[:, 0:qn], acc[C : C + 1, 0:qn])
            rb_ps = psA.tile([128, G, 512], F32, tag="sc", name="rb_ps")
            nc.tensor.matmul(rb_ps[0:C, 0, 0:qn], ones1[:], recip1[:, 0:qn])
            rbS = wk_pool.tile([C, 512], F32, tag="rbS", name="rbS")
            nc.vector.tensor_copy(rbS[:, 0:qn], rb_ps[0:C, 0, 0:qn])
            attn = wk_pool.tile([C, 512], F32R, tag="attn", name="attn")
            nc.vector.tensor_mul(attn[:, 0:qn], acc[0:C, 0:qn], rbS[:, 0:qn])
            # out = relu(wo_x @ x + wo_a @ attn + bo)
            fin = psO.tile([128, 512], F32, tag="acc", name="fin")
            nc.tensor.matmul(
                fin[0:C, 0:qn], WOX[:], XA[:, q0 : q0 + qn],
                start=True, stop=False,
            )
            nc.tensor.matmul(
                fin[0:C, 0:qn], WOA[:], attn[:, 0:qn],
                start=False, stop=True,
            )
            outs = wk_pool.tile([C, 512], F32, tag="outs", name="outs")
            nc.vector.tensor_scalar_max(outs[:, 0:qn], fin[0:C, 0:qn], 0.0)
            nc.sync.dma_start(out_d.ap()[:, q0 : q0 + qn], outs[:, 0:qn])

        # Flat software-pipelined emission over t = (chunk, j).  Each j emits
        # the mm1 trio and the two exp halves; DoubleRow mm2 pairs for a
        # completed super-step (2 j's) are drained with ~1-step lag so the PE
        # FIFO never waits on a just-issued exp.
        NT = len(QCHUNKS) * NJ
        accs = [None] * len(QCHUNKS)
        pend_pairs = []   # (qi, jj, p, st2_tile, qn)
        pend_fin = None
        vr_next = 0

        def emit_pair(qi, jj, p, st2, qn):
            kt = 6 * jj + 2 * p
            r, i = divmod(kt, 4)
            nc.tensor.matmul(
                accs[qi][0 : C + 1, 0:qn],
                VTr[r][:, i : i + 2, 0 : C + 1],
                st2[:, 2 * p : 2 * p + 2, 0:qn],
                start=(jj == 0 and p == 0),
                stop=(jj == NJJ - 1 and p == 2),
                perf_mode=mybir.MatmulPerfMode.DoubleRow,
            )

        st2_cur = None
        for t in range(NT):
            qi, j = divmod(t, NJ)
            jj = j // 2
            q0, qn = QCHUNKS[qi]
            if j == 0:
                accs[qi] = psO.tile([128, 512], F32, tag="acc", name="acc")
            if qi == 0:
                # remaining k-conv blocks: block (p, B) feeds mm1 from
                # j = 6p+3B; emit 4 j's ahead
                KBLK = {2: (1, 0), 5: (1, 1), 8: (2, 0), 11: (2, 1),
                        14: (3, 0), 17: (3, 1)}
                if j in KBLK:
                    k_block(*KBLK[j])
                # remaining q-conv chunks, well before their chunk starts
                if j in (2, 4, 6, 8):
                    q_chunk(j // 2)
            # keep v-conv a little ahead of the mm2 consumer on chunk 0
            while vr_next < NVR and (qi > 0 or 4 * vr_next <= 3 * j + 6):
                v_round(vr_next)
                vr_next += 1
            sc = psA.tile([128, G, 512], F32, tag="sc")
            jq, jjq = divmod(j, NJ // NQT)
            for g in range(G):
                nc.tensor.matmul(
                    sc[:, g, 0:qn],
                    KRq[jq][32 * g : 32 * g + 32, jjq, :],
                    QRc[qi][32 * g : 32 * g + 32, 0:qn],
                )
            if j % 2 == 0:
                st2_cur = stp.tile([128, 6, 512], F8, tag="st")
            g0 = 3 * (j % 2)
            rc = (R_SPLIT * qn) // 512
            nc.scalar.activation(
                st2_cur[:, g0 : g0 + 3, 0:rc],
                sc[:, :, 0:rc],
                mybir.ActivationFunctionType.Exp,
                bias=BIASM2[:],
            )
            nc.vector.tensor_scalar(
                st2_cur[:, g0 : g0 + 3, rc:qn].bitcast(U8),
                sc[:, :, rc:qn],
                float(SCHR_A), float(SCHR_B),
                mybir.AluOpType.mult, mybir.AluOpType.add,
            )
            if j % 2 == 1:
                for p in range(3):
                    pend_pairs.append((qi, jj, p, st2_cur, qn))
            # drain mm2 pairs with a ~1-step lag (do not emit pairs of the
            # super-step whose exp was just issued this t)
            budget = 2 if j % 2 == 0 else 1
            while pend_pairs and budget > 0:
                pqi, pjj, pp, pst2, pqn = pend_pairs[0]
                if pqi == qi and pjj == jj:
                    break
                emit_pair(*pend_pairs.pop(0))
                budget -= 1
            if j == 4 and pend_fin is not None:
                finalize(*pend_fin)
                pend_fin = None
            if j == NJ - 1:
                pend_fin = (accs[qi], q0, qn)
        for pair in pend_pairs:
            emit_pair(*pair)
        finalize(*pend_fin)

    nc.compile()
    return nc


_NC_CACHE = None


def _get_nc():
    global _NC_CACHE
    if _NC_CACHE is None:
        _NC_CACHE = build_bass()
    return _NC_CACHE


def make_in_maps(x, fusionmap, wq, bq, wk, bk, wa, ba, wo, bo):
    x = np.asarray(x, np.float32)
    fm = np.asarray(fusionmap, np.float32)
    xf = x.reshape(N, C, HW)
    fmf = fm.reshape(N, FC, HW)
    ones_hw = np.ones((1, HW), np.float32)
    wq_aug = np.concatenate(
        [np.asarray(wq).T, np.asarray(bq)[None, :]], 0
    ).astype(np.float32)
    wk_aug = np.concatenate(
        [np.asarray(wk).T, np.asarray(bk)[None, :]], 0
    ).astype(ml_dtypes.bfloat16)
    # [wa^T | 0; ba | 1]: columns C..2C-1 evaluate to exactly 1.0 after the
    # conv (ones row of fm_aug x ones), giving mm2 its denominator columns.
    wa_blk = np.concatenate([np.asarray(wa).T, np.asarray(ba)[None, :]], 0)
    ones_blk = np.concatenate(
        [np.zeros((FC, 1), np.float32), np.ones((1, 1), np.float32)], 0
    )
    wa_aug = np.concatenate([wa_blk, ones_blk], 1).astype(ml_dtypes.bfloat16)
    wo = np.asarray(wo, np.float32)
    wox_aug = np.concatenate(
        [wo[:, :C].T, np.asarray(bo)[None, :]], 0
    ).astype(np.float32)
    woa_t = np.ascontiguousarray(wo[:, C:].T).astype(np.float32)

    in_maps = []
    for core in range(NCORES):
        n, c = divmod(core, 4)
        x_chunk = xf[n][:, c * QPC : (c + 1) * QPC]
        x_aug = np.concatenate([x_chunk, ones_hw[:, :QPC]], 0)
        fm_aug = np.concatenate([fmf[n], ones_hw], 0).astype(ml_dtypes.bfloat16)
        in_maps.append(
            {
                "x_aug": np.ascontiguousarray(x_aug),
                "fm_aug": np.ascontiguousarray(fm_aug),
                "wq_aug": wq_aug,
                "wk_aug": wk_aug,
                "wa_aug": wa_aug,
                "wox_aug": wox_aug,
                "woa_t": woa_t,
            }
        )
    return in_maps


def run(in_maps, trace=False, tmpdir=None):
    nc = _get_nc()
    return bass_utils.run_bass_kernel_spmd(
        nc,
        in_maps,
        core_ids=list(range(NCORES)),
        trace=trace,
        tmpdir=tmpdir,
    )


def kernel(**inputs):
    in_maps = make_in_maps(**inputs)
    res = run(in_maps)
    out = np.empty((N, C, HW), np.float32)
    for core in range(NCORES):
        n, c = divmod(core, 4)
        out[n][:, c * QPC : (c + 1) * QPC] = res.results[core]["out_c"]
    return out.reshape(N, C, H, W)


if __name__ == "__main__":
    import reference

    inputs = {k: np.asarray(v) for k, v in reference.setup_inputs().items()}
    got = kernel(**inputs)
    print("kernel output", got.shape, got.dtype)
